# revision 16
# baseline (speedup 1.0000x reference)
"""Trainium2 Bass kernel for ColumnMixedPrecisionLinear (v13, out.T layout).

Computes out[b,s,o] = bias[o] + sum_i x_i[b,s,:] @ (wq_i * s_i[:,None]).T
where x is [4, 2048, 4096] fp32, wq_i are [4096, 1024] int8 slices of the
weight along the input dim, s_i are per-output-channel scales.

Strategy: data-parallel over tokens across 8 NeuronCores. Each core gets
1024 tokens of x and the full weights, computes its output shard
TRANSPOSED (outT[o, t]), and the host transposes back and adds bias.

Why out.T: with o on PSUM partitions, the per-channel scales become
per-PARTITION scalars, so dequantization moves into the PSUM drain
(ACT `activation(Copy, scale=s)`), applied per slice:
  outT[o, t] = sum_i s_i[o] * (wq_i @ x_i.T)[o, t]
The weights stream RAW int8 -> bf16 (exact) straight into matmuls — no
dequant multiply and no 4MB broadcast-scale tensor on the critical path,
which was most of the measured lead-in. Per (o-tile, slice): 8
accumulating matmuls [128d,128o].T @ [128d,512t] into one of 8 PSUM
banks; 4 scaled ACT drains + 3 adds (DVE/GpSimd) combine the slices.

PE floor: 2048 matmuls x 512 cols @ 2.4 GHz x 1 col/cycle ~ 444 us.
"""

import numpy as np
import ml_dtypes

import concourse.mybir as mybir
import concourse.tile as tile
from concourse import bacc
from concourse.bass_utils import run_bass_kernel_spmd

P = 128
N_CORES = 8
B, S = 4, 2048
D_IN_SLICE = 1024
N_SLICES = 4
D = D_IN_SLICE * N_SLICES      # 4096 contraction dim
O = 4096                       # out features
T = (B * S) // N_CORES         # 1024 tokens per core

T_CHUNK = 512
T_CHUNKS = T // T_CHUNK        # 2
D_BLKS = D // P                # 32
D_BLKS_SLICE = D_IN_SLICE // P # 8
O_CHUNK = 512
O_CHUNKS = O // O_CHUNK        # 8
O_TILES = O // P               # 32

BF16 = mybir.dt.bfloat16
FP32 = mybir.dt.float32
INT8 = mybir.dt.int8

VER = 14  # bumped per kernel-body revision to dodge stale NEFF cache hits


def build_nc():
    nc = bacc.Bacc(None, target_bir_lowering=False)
    vtag = nc.dram_tensor("vtag", [1, VER + 1], FP32, kind="ExternalOutput")

    # packed bf16 x.T: [p(d_low), b(d_blk), t] flattened
    xb_in = nc.dram_tensor("xb", [P, D_BLKS * T], BF16, kind="ExternalInput")
    # packed int8 W: [p, c*16384 + b*512 + ol]  (o-chunk-major, as before)
    wp_in = nc.dram_tensor("wp", [P, O_CHUNKS * D_BLKS * O_CHUNK], INT8,
                           kind="ExternalInput")
    # drain scales: sIJ[p, i, og] = s_i[og*128 + p]
    s_in = nc.dram_tensor("sij", [P, N_SLICES, O_TILES], FP32,
                          kind="ExternalInput")
    outT = nc.dram_tensor("outT", [O, T], BF16, kind="ExternalOutput")

    with tile.TileContext(nc) as tc:
        with (
            tc.tile_pool(name="const", bufs=1) as const,
            tc.tile_pool(name="xres", bufs=1) as xres,
            tc.tile_pool(name="wt", bufs=2) as wt_pool,
            tc.tile_pool(name="apool", bufs=2) as apool,
            tc.tile_pool(name="tpool", bufs=2) as tpool,
            tc.tile_pool(name="ostage", bufs=4) as ostage,
            tc.tile_pool(name="psm", bufs=8, space="PSUM") as psm,
        ):
            vt = const.tile([1, VER + 1], FP32)
            nc.vector.memset(vt[:], 0.0)
            nc.sync.dma_start(vtag[:], vt[:])

            sij = const.tile([P, N_SLICES, O_TILES], FP32)
            nc.sync.dma_start(sij[:], s_in[:])

            # x.T resident [128d, 32blk, 1024t]; 4 loads by slice so the
            # first o-tile's slice-0 matmuls only wait on 2.1MB
            xs = xres.tile([P, D_BLKS, T], BF16)
            for g in range(N_SLICES):
                nc.sync.dma_start(
                    xs[:, g * D_BLKS_SLICE:(g + 1) * D_BLKS_SLICE, :],
                    xb_in[:, g * D_BLKS_SLICE * T:(g + 1) * D_BLKS_SLICE * T]
                    .rearrange("p (b t) -> p b t", b=D_BLKS_SLICE),
                )

            wt = None
            for og in range(O_TILES):
                c, sub = og // 4, (og % 4) * P
                if sub == 0:
                    wt = wt_pool.tile([P, D_BLKS, O_CHUNK], BF16, tag="wt",
                                      name=f"wt{c}")
                    # raw int8 -> bf16 cast DMA (SWDGE ring), exact values
                    nc.gpsimd.dma_start(
                        wt[:],
                        wp_in[:, c * D_BLKS * O_CHUNK:
                              (c + 1) * D_BLKS * O_CHUNK]
                        .rearrange("p (b ol) -> p b ol", b=D_BLKS),
                    )

                ats = []
                for i in range(N_SLICES):
                    for tc_ in range(T_CHUNKS):
                        ps = psm.tile([P, T_CHUNK], FP32, tag="ps",
                                      name=f"ps{og}_{i}_{tc_}")
                        for k in range(D_BLKS_SLICE):
                            db = i * D_BLKS_SLICE + k
                            nc.tensor.matmul(
                                ps[:],
                                wt[:, db, sub:sub + P],
                                xs[:, db, tc_ * T_CHUNK:(tc_ + 1) * T_CHUNK],
                                start=(k == 0),
                                stop=(k == D_BLKS_SLICE - 1),
                            )
                        # dequant at drain: ACT per-partition scale s_i[o]
                        at = apool.tile([P, T_CHUNK], BF16, tag=f"a{i}_{tc_}",
                                        name=f"at{og}_{i}_{tc_}")
                        nc.scalar.activation(
                            at[:], ps[:], mybir.ActivationFunctionType.Copy,
                            scale=sij[:, i, og:og + 1],
                        )
                        ats.append(at)

                for tc_ in range(T_CHUNKS):
                    t01 = tpool.tile([P, T_CHUNK], BF16, tag=f"t01_{tc_}",
                                     name=f"t01_{og}_{tc_}")
                    nc.vector.tensor_tensor(
                        t01[:], ats[0 * T_CHUNKS + tc_][:],
                        ats[1 * T_CHUNKS + tc_][:], mybir.AluOpType.add)
                    t23 = tpool.tile([P, T_CHUNK], BF16, tag=f"t23_{tc_}",
                                     name=f"t23_{og}_{tc_}")
                    nc.gpsimd.tensor_tensor(
                        t23[:], ats[2 * T_CHUNKS + tc_][:],
                        ats[3 * T_CHUNKS + tc_][:], mybir.AluOpType.add)
                    ob = ostage.tile([P, T_CHUNK], BF16, tag="ob",
                                     name=f"ob{og}_{tc_}")
                    nc.vector.tensor_tensor(
                        ob[:], t01[:], t23[:], mybir.AluOpType.add)
                    nc.sync.dma_start(
                        outT[og * P:(og + 1) * P,
                             tc_ * T_CHUNK:(tc_ + 1) * T_CHUNK],
                        ob[:],
                    )
    nc.compile()
    return nc


_NC_CACHE = None


def _get_nc():
    global _NC_CACHE
    if _NC_CACHE is None:
        _NC_CACHE = build_nc()
    return _NC_CACHE


def _prep_inputs(x, wqs, ss):
    xf = np.asarray(x, dtype=np.float32).reshape(B * S, D)

    # Wfull[d, o] = wq_{d // 1024}[o, d % 1024]
    wfull = np.empty((D, O), dtype=np.int8)
    for i in range(N_SLICES):
        wfull[i * D_IN_SLICE:(i + 1) * D_IN_SLICE, :] = np.asarray(wqs[i]).T
    # [d, o] -> [b, p, c, ol] -> [p, c, b, ol] -> [p, c*16384 + b*512 + ol]
    wp = np.ascontiguousarray(
        wfull.reshape(D_BLKS, P, O_CHUNKS, O_CHUNK).transpose(1, 2, 0, 3)
    ).reshape(P, O_CHUNKS * D_BLKS * O_CHUNK)

    # sIJ[p, i, og] = s_i[og*128 + p]
    sij = np.ascontiguousarray(
        np.stack([np.asarray(s, dtype=np.float32).reshape(O_TILES, P).T
                  for s in ss], axis=1)
    )  # [128, 4, 32]

    in_maps = []
    for cid in range(N_CORES):
        xc = xf[cid * T:(cid + 1) * T]  # [1024, 4096]
        # x.T packed: [p, b, t] = x[t, b*128+p]
        xbc = np.ascontiguousarray(
            xc.reshape(T, D_BLKS, P).transpose(2, 1, 0)
            .astype(ml_dtypes.bfloat16)
        ).reshape(P, D_BLKS * T)
        in_maps.append({"xb": xbc, "wp": wp, "sij": sij})
    return in_maps


def run_on_hw(x, wqs, ss, bias, **spmd_kwargs):
    """Run and return (out_full [B,S,O] fp32, BassKernelResults)."""
    nc = _get_nc()
    in_maps = _prep_inputs(x, wqs, ss)
    res = run_bass_kernel_spmd(nc, in_maps, core_ids=list(range(N_CORES)),
                               **spmd_kwargs)
    out = np.empty((B * S, O), dtype=np.float32)
    for cid, r in enumerate(res.results):
        out[cid * T:(cid + 1) * T, :] = r["outT"].astype(np.float32).T
    out += np.asarray(bias, dtype=np.float32)
    return np.ascontiguousarray(out.reshape(B, S, O)), res


def kernel(x, wq0, s0, wq1, s1, wq2, s2, wq3, s3, bias):
    out, _ = run_on_hw(x, [wq0, wq1, wq2, wq3], [s0, s1, s2, s3], bias)
    return out


# revision 17
# speedup vs baseline: 1.0264x; 1.0264x over previous
"""Trainium2 Bass kernel for ColumnMixedPrecisionLinear (v14 == v5 config, best measured).

Computes out[b,s,o] = bias[o] + sum_i x_i[b,s,:] @ (wq_i * s_i[:,None]).T
where x is [4, 2048, 4096] fp32, wq_i are [4096, 1024] int8 slices of the
weight along the input dim, s_i are per-output-channel scales.

Strategy: data-parallel over tokens across 8 NeuronCores. Each core gets
1024 tokens of x (flattened [8192, 4096]) and the full weights, and computes
its [1024, 4096] output shard. No cross-device reduction needed.

Design — keep the PE streaming back-to-back, nothing else on the critical
path (PE floor: 2048 matmuls x 512 cols @ 2.4 GHz ~ 445 us):
  - x pre-transposed/packed/cast to bf16 on the HOST:
    xb[p, j, b, tl] = bf16(x[t=j*128+tl, d=b*128+p]); one 1MB DMA per
    j-tile. No device-side transposes.
  - W pre-transposed/packed on the host, int8:
    wp[p, c*16384 + b*512 + ol] = Wfull[d=b*128+p, o=c*512+ol]; per o-chunk
    one contiguous 16KB/partition SWDGE DMA with int8->bf16 cast (exact).
  - Scales packed PER CHUNK (sc[p, c, i, ol] = s_i[c*512+ol]) so chunk 0's
    dequant only waits on a 512KB load, not the whole scale tensor (v3
    lost 35us of lead-in to that).
  - Dequant: DVE broadcast multiply per (chunk, slice).
  - Main loop per (ochunk c, ttile j): 32 accumulating matmuls
    [128d,128t] @ [128d,512o] into one PSUM bank; ACT drains psum -> bf16;
    per-(c,j) 128KB output stores keep the tail short.
  - bias added on the HOST (output-linear); out stored bf16, upcast on host.
"""

import numpy as np
import ml_dtypes

import concourse.bass as bass
import concourse.mybir as mybir
import concourse.tile as tile
from concourse import bacc
from concourse.bass_utils import run_bass_kernel_spmd

P = 128
N_CORES = 8
B, S = 4, 2048
D_IN_SLICE = 1024
N_SLICES = 4
D = D_IN_SLICE * N_SLICES      # 4096 contraction dim
O = 4096                       # out features
T = (B * S) // N_CORES         # 1024 tokens per core

T_TILES = T // P               # 8
D_BLKS = D // P                # 32
D_BLKS_SLICE = D_IN_SLICE // P # 8
O_CHUNK = 512
O_CHUNKS = O // O_CHUNK        # 8

BF16 = mybir.dt.bfloat16
FP32 = mybir.dt.float32
INT8 = mybir.dt.int8

VER = 15  # bumped per kernel-body revision to dodge stale NEFF cache hits


def build_nc():
    nc = bacc.Bacc(None, target_bir_lowering=False)
    vtag = nc.dram_tensor("vtag", [1, VER + 1], FP32, kind="ExternalOutput")

    # packed bf16 x: [p, j*4096 + b*128 + tl]
    xb_in = nc.dram_tensor("xb", [P, T_TILES * D_BLKS * P], BF16,
                           kind="ExternalInput")
    # packed int8 W: [p, c*16384 + b*512 + ol]
    wp_in = nc.dram_tensor("wp", [P, O_CHUNKS * D_BLKS * O_CHUNK], INT8,
                           kind="ExternalInput")
    # per-chunk partition-broadcast scales: [p, c, i, ol] = s_i[c*512+ol]
    sc_in = nc.dram_tensor("sc", [P, O_CHUNKS * N_SLICES, O_CHUNK], BF16,
                           kind="ExternalInput")
    out = nc.dram_tensor("out", [T, O], BF16, kind="ExternalOutput")

    with tile.TileContext(nc) as tc:
        with (
            tc.tile_pool(name="const", bufs=1) as const,
            tc.tile_pool(name="xres", bufs=1) as xres,
            tc.tile_pool(name="scp", bufs=1) as sc_pool,
            tc.tile_pool(name="wt", bufs=2) as wt_pool,
            tc.tile_pool(name="ostage", bufs=4) as ostage,
            tc.tile_pool(name="psm", bufs=8, space="PSUM") as psm,
        ):
            vt = const.tile([1, VER + 1], FP32)
            nc.vector.memset(vt[:], 0.0)
            nc.sync.dma_start(vtag[:], vt[:])

            # DMA issue on the sequencers costs ~0.6-1.2us per dma_start
            # (transfers themselves are fast) -> load x and all scales as
            # ONE DMA each, on separate HWDGE rings.
            xs = xres.tile([P, T_TILES, D_BLKS, P], BF16)
            sc_tiles = []
            for j in range(T_TILES):
                sct = sc_pool.tile([P, N_SLICES, O_CHUNK], BF16,
                                   tag=f"sc{j}", name=f"sct{j}")
                nc.sync.dma_start(
                    sct[:],
                    sc_in[:, j * N_SLICES:(j + 1) * N_SLICES, :])
                sc_tiles.append(sct)
                nc.sync.dma_start(
                    xs[:, j, :, :],
                    xb_in[:, j * D_BLKS * P:(j + 1) * D_BLKS * P]
                    .rearrange("p (b tl) -> p b tl", b=D_BLKS),
                )

            for c in range(O_CHUNKS):
                wt = wt_pool.tile([P, D_BLKS, O_CHUNK], BF16, tag="wt")
                # one int8 -> bf16 cast DMA per chunk (SWDGE ring)
                nc.gpsimd.dma_start(
                    wt[:],
                    wp_in[:, c * D_BLKS * O_CHUNK:(c + 1) * D_BLKS * O_CHUNK]
                    .rearrange("p (b ol) -> p b ol", b=D_BLKS),
                )
                # dequant: per-slice broadcast multiply on DVE (bf16, 2/cyc)
                for i in range(N_SLICES):
                    nc.vector.tensor_tensor(
                        wt[:, i * D_BLKS_SLICE:(i + 1) * D_BLKS_SLICE, :],
                        wt[:, i * D_BLKS_SLICE:(i + 1) * D_BLKS_SLICE, :],
                        sc_tiles[c][:, i, None, :]
                        .to_broadcast((P, D_BLKS_SLICE, O_CHUNK)),
                        mybir.AluOpType.mult,
                    )

                for j in range(T_TILES):
                    ps = psm.tile([P, O_CHUNK], FP32, tag="ps")
                    for db in range(D_BLKS):
                        nc.tensor.matmul(
                            ps[:],
                            xs[:, j, db, :],
                            wt[:, db, :],
                            start=(db == 0),
                            stop=(db == D_BLKS - 1),
                        )
                    # drain on ACT: psum fp32 -> sbuf bf16
                    ob = ostage.tile([P, O_CHUNK], BF16, tag="ob")
                    nc.scalar.activation(
                        ob[:], ps[:], mybir.ActivationFunctionType.Copy
                    )
                    nc.sync.dma_start(
                        out[j * P:(j + 1) * P, c * O_CHUNK:(c + 1) * O_CHUNK],
                        ob[:],
                    )
    nc.compile()
    return nc


_NC_CACHE = None


def _get_nc():
    global _NC_CACHE
    if _NC_CACHE is None:
        _NC_CACHE = build_nc()
    return _NC_CACHE


def _prep_inputs(x, wqs, ss):
    xf = np.asarray(x, dtype=np.float32).reshape(B * S, D)

    # Wfull[d, o] = wq_{d // 1024}[o, d % 1024]
    wfull = np.empty((D, O), dtype=np.int8)
    for i in range(N_SLICES):
        wfull[i * D_IN_SLICE:(i + 1) * D_IN_SLICE, :] = np.asarray(wqs[i]).T
    # [d, o] -> [b, p, c, ol] -> [p, c, b, ol] -> [p, c*16384 + b*512 + ol]
    wp = np.ascontiguousarray(
        wfull.reshape(D_BLKS, P, O_CHUNKS, O_CHUNK).transpose(1, 2, 0, 3)
    ).reshape(P, O_CHUNKS * D_BLKS * O_CHUNK)

    # sc[p, c, i, ol] = s_i[c*512 + ol]
    sstack = np.stack([np.asarray(s, dtype=np.float32) for s in ss])  # [4, O]
    sc = np.ascontiguousarray(
        np.broadcast_to(
            sstack.reshape(N_SLICES, O_CHUNKS, O_CHUNK)
            .transpose(1, 0, 2)
            .astype(ml_dtypes.bfloat16)[None],
            (P, O_CHUNKS, N_SLICES, O_CHUNK),
        )
    ).reshape(P, O_CHUNKS * N_SLICES, O_CHUNK)

    in_maps = []
    for c in range(N_CORES):
        xc = xf[c * T:(c + 1) * T]  # [1024, 4096]
        # [t, d] = [(j tl), (b p)] -> [p, j, b, tl]
        xbc = np.ascontiguousarray(
            xc.reshape(T_TILES, P, D_BLKS, P)
            .transpose(3, 0, 2, 1)
            .astype(ml_dtypes.bfloat16)
        ).reshape(P, T_TILES * D_BLKS * P)
        in_maps.append({"xb": xbc, "wp": wp, "sc": sc})
    return in_maps


def run_on_hw(x, wqs, ss, bias, **spmd_kwargs):
    """Run and return (out_full [B,S,O] fp32, BassKernelResults)."""
    nc = _get_nc()
    in_maps = _prep_inputs(x, wqs, ss)
    res = run_bass_kernel_spmd(nc, in_maps, core_ids=list(range(N_CORES)),
                               **spmd_kwargs)
    out = np.concatenate([r["out"] for r in res.results], axis=0)
    out = out.astype(np.float32) + np.asarray(bias, dtype=np.float32)
    return np.ascontiguousarray(out.reshape(B, S, O)), res


def kernel(x, wq0, s0, wq1, s1, wq2, s2, wq3, s3, bias):
    out, _ = run_on_hw(x, [wq0, wq1, wq2, wq3], [s0, s1, s2, s3], bias)
    return out
